# revision 5
# baseline (speedup 1.0000x reference)
"""Trainium2 Bass kernel for nn_LogicRecursiveNN.

Reference computation (B=512, N_ENT=2000, E=512, T=512, DEPTH=6, N_TREES=2):
  x = ent_emb[leaf_idx]                         # [B, 2, 64, E]
  6x binary-tree reduction with shared MLP:     # pairs -> two2one(2E->4E->E), ReLU/ReLU
  one2one MLP on roots (E->4E->E), ReLU/Tanh
  head on concat(th_emb, roots):  (T+2E)->E->E/2->E/4->1, ReLU x3 + Sigmoid

Strategy:
  - Data parallel over batch: 64 batch elements per core, 8 cores, no collectives.
  - Activations kept feature-major ("X^T": features on SBUF partitions, tree
    nodes on the free dim).  Matmuls put the weights stationary:
        out[M=feat_chunk, N=node_cols] = w[K,M].T @ X^T[K, N]
    so each level's output is already feature-major for the next level.
  - Pair-concatenation between levels is free: the (even, odd) node columns of
    a level's output are de-interleaved into feature chunks [0:E) / [E:2E) of
    the next level's input during the PSUM->SBUF ReLU copy.
  - Leaf gather: indirect DMA of ent_emb rows (row-major) + PE transpose to
    feature-major.
  - fp16 operands everywhere (PE multiplies fp16 exactly at FP22, accumulates
    fp32 in PSUM; measured end-to-end rel err ~3e-4), 1 cycle/row on the PE.
"""

import numpy as np

import concourse.bass as bass
import concourse.mybir as mybir
import concourse.tile as tile
from concourse.bass import IndirectOffsetOnAxis
from concourse.bass_utils import run_bass_kernel_spmd
from concourse.masks import make_identity
from concourse.vector_clock import ScopedClock

F16 = mybir.dt.float16
F32 = mybir.dt.float32
I32 = mybir.dt.int32
AF = mybir.ActivationFunctionType

B, N_ENT, E, T, N_TREES, N_LEAVES = 512, 2000, 512, 512, 2, 64
N_CORES = 8
B_SH = B // N_CORES              # 64 batch elements per core
LEAVES = B_SH * N_TREES * N_LEAVES   # 8192 leaf slots per core
G_GROUPS = LEAVES // 128             # 64 gather groups of 128 leaves
P = 128


class _SplitDrainTileContext(tile.TileContext):
    """TileContext that splits multi-semaphore waits after scheduling.

    The neuronxcc walrus in this container rejects any instruction carrying
    more than one sync-wait command ("Too many sync wait commands",
    CoreV3GenImpl.cpp setupSyncWait).  Tile freely attaches several waits to
    one instruction, so after scheduling we hoist all but the last wait of
    each instruction onto standalone InstEventSemaphore instructions spliced
    just before it on the same engine (engines execute their stream in
    order, so the semantics are identical, merely more conservative for
    DMA-queue instructions).
    """

    def _drain_and_barrier(self, tick_clock, wait_clock):
        super()._drain_and_barrier(tick_clock, wait_clock)
        import bass_rust

        nc = self.nc
        for bb_wrap in nc.m.functions[0].blocks:
            insts = bb_wrap.instructions
            out = []
            changed = False
            for inst in insts:
                si = inst.sync_info
                waits = list(si.on_wait) if si is not None else []
                if len(waits) > 1:
                    changed = True
                    for w in waits[:-1]:
                        ev = mybir.InstEventSemaphore(
                            name=f"I-wsplit-{nc.next_id()}", ins=[], outs=[]
                        )
                        ev.engine = inst.engine
                        ev.sync_info = bass_rust.SyncInfo(
                            on_wait=[w], on_update=[]
                        )
                        nc.register_instruction(ev)
                        out.append(ev)
                    si.on_wait = [waits[-1]]
                out.append(inst)
            if changed:
                bb_wrap.instructions = out


def _build_nc():
    nc = bass.Bass()

    # ---- DRAM I/O (host passes pre-laid-out arrays, see kernel()) ----
    d_idx = nc.dram_tensor("idx", [P, G_GROUPS], I32, kind="ExternalInput")
    d_emb = nc.dram_tensor("emb", [N_ENT, E], F16, kind="ExternalInput")
    d_w1 = nc.dram_tensor("w1", [P, 8, 2048], F16, kind="ExternalInput")
    d_w2 = nc.dram_tensor("w2", [P, 16, 512], F16, kind="ExternalInput")
    d_o1 = nc.dram_tensor("o1", [P, 4, 2048], F16, kind="ExternalInput")
    d_o2 = nc.dram_tensor("o2", [P, 16, 512], F16, kind="ExternalInput")
    d_h1 = nc.dram_tensor("h1", [P, 12, 512], F16, kind="ExternalInput")
    d_h2 = nc.dram_tensor("h2", [P, 4, 256], F16, kind="ExternalInput")
    d_h3 = nc.dram_tensor("h3", [P, 2, 128], F16, kind="ExternalInput")
    d_h4 = nc.dram_tensor("h4", [P, 1], F16, kind="ExternalInput")
    d_b1 = nc.dram_tensor("b1", [P, 16], F32, kind="ExternalInput")
    d_b2 = nc.dram_tensor("b2", [P, 4], F32, kind="ExternalInput")
    d_ob1 = nc.dram_tensor("ob1", [P, 16], F32, kind="ExternalInput")
    d_ob2 = nc.dram_tensor("ob2", [P, 4], F32, kind="ExternalInput")
    d_hb1 = nc.dram_tensor("hb1", [P, 4], F32, kind="ExternalInput")
    d_hb2 = nc.dram_tensor("hb2", [P, 2], F32, kind="ExternalInput")
    d_hb3 = nc.dram_tensor("hb3", [P, 1], F32, kind="ExternalInput")
    d_hb4 = nc.dram_tensor("hb4", [1, 1], F32, kind="ExternalInput")
    d_th = nc.dram_tensor("th", [P, 4], F16, kind="ExternalInput")
    d_out = nc.dram_tensor("out", [1, B_SH], F32, kind="ExternalOutput")

    with _SplitDrainTileContext(nc) as tc:
        with (
            tc.tile_pool(name="const", bufs=1) as cpool,
            tc.tile_pool(name="work", bufs=1) as wpool,
            tc.tile_pool(name="psum", bufs=1, space="PSUM") as ppool,
        ):
            # ---- resident constants ----
            idx_s = cpool.tile([P, G_GROUPS], I32, name="idx_s")
            nc.sync.dma_start(idx_s[:], d_idx[:])
            w1_s = cpool.tile([P, 8, 2048], F16, name="w1_s")
            nc.sync.dma_start(w1_s[:], d_w1[:])
            w2_s = cpool.tile([P, 16, 512], F16, name="w2_s")
            nc.sync.dma_start(w2_s[:], d_w2[:])
            o1_s = cpool.tile([P, 4, 2048], F16, name="o1_s")
            nc.sync.dma_start(o1_s[:], d_o1[:])
            o2_s = cpool.tile([P, 16, 512], F16, name="o2_s")
            nc.sync.dma_start(o2_s[:], d_o2[:])
            h1_s = cpool.tile([P, 12, 512], F16, name="h1_s")
            nc.sync.dma_start(h1_s[:], d_h1[:])
            h2_s = cpool.tile([P, 4, 256], F16, name="h2_s")
            nc.sync.dma_start(h2_s[:], d_h2[:])
            h3_s = cpool.tile([P, 2, 128], F16, name="h3_s")
            nc.sync.dma_start(h3_s[:], d_h3[:])
            h4_s = cpool.tile([P, 1], F16, name="h4_s")
            nc.sync.dma_start(h4_s[:], d_h4[:])
            b1_s = cpool.tile([P, 16], F32, name="b1_s")
            nc.sync.dma_start(b1_s[:], d_b1[:])
            b2_s = cpool.tile([P, 4], F32, name="b2_s")
            nc.sync.dma_start(b2_s[:], d_b2[:])
            ob1_s = cpool.tile([P, 16], F32, name="ob1_s")
            nc.sync.dma_start(ob1_s[:], d_ob1[:])
            ob2_s = cpool.tile([P, 4], F32, name="ob2_s")
            nc.sync.dma_start(ob2_s[:], d_ob2[:])
            hb1_s = cpool.tile([P, 4], F32, name="hb1_s")
            nc.sync.dma_start(hb1_s[:], d_hb1[:])
            hb2_s = cpool.tile([P, 2], F32, name="hb2_s")
            nc.sync.dma_start(hb2_s[:], d_hb2[:])
            hb3_s = cpool.tile([P, 1], F32, name="hb3_s")
            nc.sync.dma_start(hb3_s[:], d_hb3[:])
            hb4_s = cpool.tile([1, 1], F32, name="hb4_s")
            nc.sync.dma_start(hb4_s[:], d_hb4[:])
            th_s = cpool.tile([P, 4], F16, name="th_s")
            nc.sync.dma_start(th_s[:], d_th[:])
            ident = cpool.tile([P, P], F16, name="ident")
            make_identity(nc, ident[:])

            # ---- persistent level-input buffers (feature-major) ----
            # xt[l] holds level-l input pairs: [128, 8 K-chunks, n_pairs(l)]
            xt = {
                2: cpool.tile([P, 8, 2048], F16, name="xt2"),
                3: cpool.tile([P, 8, 1024], F16, name="xt3"),
                4: cpool.tile([P, 8, 512], F16, name="xt4"),
                5: cpool.tile([P, 8, 256], F16, name="xt5"),
                6: cpool.tile([P, 8, 128], F16, name="xt6"),
            }
            roots_s = cpool.tile([P, 4, 128], F16, name="roots_s")
            feat_s = cpool.tile([P, 12, B_SH], F16, name="feat_s")
            z1_s = cpool.tile([P, 4, B_SH], F16, name="z1_s")
            z2_s = cpool.tile([P, 2, B_SH], F16, name="z2_s")
            z3_s = cpool.tile([P, B_SH], F16, name="z3_s")
            out_s = cpool.tile([1, B_SH], F32, name="out_s")

            def two2one_tile(xin, cols, writer):
                """One col-tile of the shared two2one MLP.

                xin: [128, 8, cols] fp16 feature-major input pairs.
                writer(rc, pr_ap): consume ReLU'd mm2 PSUM chunk rc.
                """
                prs = [
                    ppool.tile([P, 512], F32, name=f"pr{rc}", tag="pr", bufs=4)[
                        :, :cols
                    ]
                    for rc in range(4)
                ]
                hbs = [None] * 16
                for hc in range(16):
                    ph = ppool.tile([P, 512], F32, name="ph", tag="ph", bufs=2)[
                        :, :cols
                    ]
                    for kc in range(8):
                        nc.tensor.matmul(
                            ph,
                            w1_s[:, kc, hc * 128 : (hc + 1) * 128],
                            xin[:, kc, :],
                            start=(kc == 0),
                            stop=(kc == 7),
                        )
                    hb = wpool.tile([P, 512], F16, name="hb", tag="hb", bufs=3)[
                        :, :cols
                    ]
                    nc.scalar.activation(
                        hb, ph, AF.Relu, bias=b1_s[:, hc : hc + 1]
                    )
                    hbs[hc] = hb
                    # skewed mm2: issue the previous chunk's 4 accum matmuls
                    # after this chunk's mm1, so the PE never waits on ACT
                    if hc > 0:
                        for rc in range(4):
                            nc.tensor.matmul(
                                prs[rc],
                                w2_s[:, hc - 1, rc * 128 : (rc + 1) * 128],
                                hbs[hc - 1],
                                start=(hc - 1 == 0),
                                stop=False,
                            )
                for rc in range(4):
                    nc.tensor.matmul(
                        prs[rc],
                        w2_s[:, 15, rc * 128 : (rc + 1) * 128],
                        hbs[15],
                        start=False,
                        stop=True,
                    )
                for rc in range(4):
                    writer(rc, prs[rc])

            def deint_writer(xnext, base):
                """ReLU+bias the mm2 output and de-interleave even/odd node
                columns into the next level's low/high feature chunks."""

                def w(rc, pr):
                    nc.scalar.activation(
                        xnext[:, rc, base : base + pr.shape[1] // 2],
                        pr[:, 0::2],
                        AF.Relu,
                        bias=b2_s[:, rc : rc + 1],
                    )
                    nc.scalar.activation(
                        xnext[:, rc + 4, base : base + pr.shape[1] // 2],
                        pr[:, 1::2],
                        AF.Relu,
                        bias=b2_s[:, rc : rc + 1],
                    )

                return w

            # ---- level 1: gather + transpose + MLP, 8 col-tiles of 512 ----
            for t in range(8):
                x1t = wpool.tile([P, 8, 512], F16, name="x1t", tag="x1", bufs=2)
                for half in range(2):
                    lvs = []
                    for g4 in range(4):
                        g = t * 8 + half * 4 + g4
                        lv = wpool.tile(
                            [P, E], F16, name=f"lv{g4}", tag="lv", bufs=8
                        )
                        nc.gpsimd.indirect_dma_start(
                            out=lv[:],
                            out_offset=None,
                            in_=d_emb[:],
                            in_offset=IndirectOffsetOnAxis(
                                ap=idx_s[:, g : g + 1], axis=0
                            ),
                        )
                        lvs.append(lv)
                    for c in range(4):
                        ps = ppool.tile(
                            [P, 512], F16, name="ptr", tag="ptr", bufs=2
                        )
                        for g4 in range(4):
                            nc.tensor.transpose(
                                ps[:, g4 * 128 : (g4 + 1) * 128],
                                lvs[g4][:, c * 128 : (c + 1) * 128],
                                ident[:],
                            )
                        nc.vector.tensor_copy(
                            out=x1t[:, c, half * 256 : (half + 1) * 256],
                            in_=ps[:, 0::2],
                        )
                        nc.vector.tensor_copy(
                            out=x1t[:, c + 4, half * 256 : (half + 1) * 256],
                            in_=ps[:, 1::2],
                        )
                two2one_tile(x1t[:], 512, deint_writer(xt[2], t * 256))

            # ---- levels 2..5 (into next buffer), level 6 (into roots) ----
            for lvl, n_tiles, cols in ((2, 4, 512), (3, 2, 512), (4, 1, 512), (5, 1, 256)):
                for t in range(n_tiles):
                    two2one_tile(
                        xt[lvl][:, :, t * cols : (t + 1) * cols],
                        cols,
                        deint_writer(xt[lvl + 1], t * cols // 2),
                    )

            def roots_writer(rc, pr):
                nc.scalar.activation(
                    roots_s[:, rc, :], pr, AF.Relu, bias=b2_s[:, rc : rc + 1]
                )

            two2one_tile(xt[6][:], 128, roots_writer)

            # ---- one2one on the 128 root columns ----
            pr2 = [
                ppool.tile([P, 512], F32, name=f"pr2_{rc}", tag="pr", bufs=4)[
                    :, :128
                ]
                for rc in range(4)
            ]
            hb2s = [None] * 16
            for hc in range(16):
                ph = ppool.tile([P, 512], F32, name="ph2", tag="ph", bufs=2)[
                    :, :128
                ]
                for kc in range(4):
                    nc.tensor.matmul(
                        ph,
                        o1_s[:, kc, hc * 128 : (hc + 1) * 128],
                        roots_s[:, kc, :],
                        start=(kc == 0),
                        stop=(kc == 3),
                    )
                hb = wpool.tile([P, 512], F16, name="hbt", tag="hb", bufs=3)[:, :128]
                nc.scalar.activation(hb, ph, AF.Relu, bias=ob1_s[:, hc : hc + 1])
                hb2s[hc] = hb
                if hc > 0:
                    for rc in range(4):
                        nc.tensor.matmul(
                            pr2[rc],
                            o2_s[:, hc - 1, rc * 128 : (rc + 1) * 128],
                            hb2s[hc - 1],
                            start=(hc - 1 == 0),
                            stop=False,
                        )
            for rc in range(4):
                nc.tensor.matmul(
                    pr2[rc],
                    o2_s[:, 15, rc * 128 : (rc + 1) * 128],
                    hb2s[15],
                    start=False,
                    stop=True,
                )

            # feat = concat(th_emb, tanh(root) for tree0, tree1)
            for c in range(4):
                nc.vector.tensor_copy(
                    out=feat_s[:, c, :],
                    in_=th_s[:, c : c + 1].to_broadcast([P, B_SH]),
                )
            for rc in range(4):
                nc.scalar.activation(
                    feat_s[:, 4 + rc, :],
                    pr2[rc][:, 0::2],
                    AF.Tanh,
                    bias=ob2_s[:, rc : rc + 1],
                )
                nc.scalar.activation(
                    feat_s[:, 8 + rc, :],
                    pr2[rc][:, 1::2],
                    AF.Tanh,
                    bias=ob2_s[:, rc : rc + 1],
                )

            # ---- head ----
            for mc in range(4):
                pz = ppool.tile([P, 512], F32, name="pz1", tag="ph", bufs=2)[
                    :, :B_SH
                ]
                for kc in range(12):
                    nc.tensor.matmul(
                        pz,
                        h1_s[:, kc, mc * 128 : (mc + 1) * 128],
                        feat_s[:, kc, :],
                        start=(kc == 0),
                        stop=(kc == 11),
                    )
                nc.scalar.activation(
                    z1_s[:, mc, :], pz, AF.Relu, bias=hb1_s[:, mc : mc + 1]
                )
            for mc in range(2):
                pz = ppool.tile([P, 512], F32, name="pz2", tag="ph", bufs=2)[
                    :, :B_SH
                ]
                for kc in range(4):
                    nc.tensor.matmul(
                        pz,
                        h2_s[:, kc, mc * 128 : (mc + 1) * 128],
                        z1_s[:, kc, :],
                        start=(kc == 0),
                        stop=(kc == 3),
                    )
                nc.scalar.activation(
                    z2_s[:, mc, :], pz, AF.Relu, bias=hb2_s[:, mc : mc + 1]
                )
            pz = ppool.tile([P, 512], F32, name="pz3", tag="ph", bufs=2)[:, :B_SH]
            for kc in range(2):
                nc.tensor.matmul(
                    pz,
                    h3_s[:, kc, :],
                    z2_s[:, kc, :],
                    start=(kc == 0),
                    stop=(kc == 1),
                )
            nc.scalar.activation(z3_s[:], pz, AF.Relu, bias=hb3_s[:, 0:1])
            po = ppool.tile([P, 512], F32, name="po", tag="ph", bufs=2)[:1, :B_SH]
            nc.tensor.matmul(po, h4_s[:], z3_s[:], start=True, stop=True)
            nc.scalar.activation(out_s[:], po, AF.Sigmoid, bias=hb4_s[:1, :1])
            nc.sync.dma_start(d_out[:], out_s[:])

    return nc


_NC_CACHE = None


def _get_nc():
    global _NC_CACHE
    if _NC_CACHE is None:
        _NC_CACHE = _build_nc()
    return _NC_CACHE


def _kxm(w, K, M):
    """[K, M] -> [128, K/128, M] (contraction dim on partitions)."""
    return np.ascontiguousarray(
        np.asarray(w).astype(np.float16).reshape(K // P, P, M).transpose(1, 0, 2)
    )


def _pbias(b, M):
    """[M] -> [128, M/128] float32 (feature-major per-partition scalars)."""
    return np.ascontiguousarray(
        np.asarray(b).astype(np.float32).reshape(M // P, P).T
    )


def kernel(
    leaf_idx,
    ent_emb,
    th_emb,
    w1,
    b1,
    w2,
    b2,
    o1,
    ob1,
    o2,
    ob2,
    h1,
    hb1,
    h2,
    hb2,
    h3,
    hb3,
    h4,
    hb4,
):
    nc = _get_nc()

    shared = {
        "emb": np.ascontiguousarray(np.asarray(ent_emb).astype(np.float16)),
        "w1": _kxm(w1, 2 * E, 4 * E),
        "w2": _kxm(w2, 4 * E, E),
        "o1": _kxm(o1, E, 4 * E),
        "o2": _kxm(o2, 4 * E, E),
        "h1": _kxm(h1, T + N_TREES * E, E),
        "h2": _kxm(h2, E, E // 2),
        "h3": _kxm(h3, E // 2, E // 4),
        "h4": np.ascontiguousarray(np.asarray(h4).astype(np.float16)),  # [128,1]
        "b1": _pbias(b1, 4 * E),
        "b2": _pbias(b2, E),
        "ob1": _pbias(ob1, 4 * E),
        "ob2": _pbias(ob2, E),
        "hb1": _pbias(hb1, E),
        "hb2": _pbias(hb2, E // 2),
        "hb3": _pbias(hb3, E // 4),
        "hb4": np.asarray(hb4).astype(np.float32).reshape(1, 1),
        "th": np.ascontiguousarray(
            np.asarray(th_emb).astype(np.float16).reshape(4, P).T
        ),
    }
    li = np.asarray(leaf_idx).astype(np.int32).reshape(B, N_TREES * N_LEAVES)
    in_maps = []
    for c in range(N_CORES):
        flat = li[c * B_SH : (c + 1) * B_SH].reshape(-1)  # [8192]
        idx = np.ascontiguousarray(flat.reshape(G_GROUPS, P).T)  # [128, 64]
        in_maps.append({**shared, "idx": idx})

    res = run_bass_kernel_spmd(
        nc, in_maps, core_ids=list(range(N_CORES)), trace=False
    )
    out = np.concatenate(
        [res.results[c]["out"].reshape(B_SH) for c in range(N_CORES)]
    )
    return out.reshape(B, 1).astype(np.float32)
